# revision 10
# baseline (speedup 1.0000x reference)
"""Trainium2 Bass kernel for nn_DNC_65189013619263.

6-layer transformer (D=1024, H=16, FF=4096, T=2048, B=2) + StateBank
content-addressed read + tied LM head over V=32000, run SPMD on 8
NeuronCores.

Sharding: 8-way over tokens (core c -> batch c//4, tokens (c%4)*512..+512).
Per layer, each core projects q/k/v for its own 512 tokens and the k/v
shards are AllGather'd within each batch's 4-core group; attention, FFN,
StateBank and the LM head are then fully local. Weights are replicated
per core (bf16), activations kept in f32 in SBUF with bf16 matmul
operands.
"""

import os
import sys

sys.path.insert(0, "/opt/trn_rl_repo")

import numpy as np
import ml_dtypes

import concourse.bass as bass
import concourse.bacc as bacc
import concourse.tile as tile
import concourse.mybir as mybir
from concourse.bass import ds
from concourse.masks import make_identity

V, D, L, H, SLOTS, FF = 32000, 1024, 6, 16, 4096, 4096
DH = D // H
B, T, NCORES, TLOC = 2, 2048, 8, 512
P = 128
DT = D // P            # 8 feature tiles
JT = TLOC // P         # 4 local token tiles
KT = T // P            # 16 key tiles (full batch)
FT = FF // P           # 32 ff tiles
ST = SLOTS // P        # 32 slot tiles
NH = D // 512          # 2 psum halves of D
EPS = 1e-8

BF = mybir.dt.bfloat16
F32 = mybir.dt.float32
I32 = mybir.dt.int32
MUL = mybir.AluOpType.mult
ADD = mybir.AluOpType.add
SUB = mybir.AluOpType.subtract
AF = mybir.ActivationFunctionType
GROUPS = [[0, 1, 2, 3], [4, 5, 6, 7]]

L_RUN = int(os.environ.get("DNC_LAYERS", str(L)))
STAGE = os.environ.get("DNC_STAGE", "full")  # emb | layers | sb | full
SUBSTAGE = os.environ.get("DNC_SUB", "C")  # truncate layer after phase A/B (debug)


# ----------------------------------------------------------------------------
# bass program
# ----------------------------------------------------------------------------

class _Ctx:
    """Holds persistent tiles and pools during program build."""
    pass


def _transpose_to_feat(nc, cx, pt, xf_src, dst_bf):
    """Token-major [128, JT, D] f32 -> feature-major bf16 [128, DT, TLOC]."""
    for j in range(JT):
        for d in range(DT):
            pst = pt.tile([P, P], F32, tag="pst")
            nc.tensor.transpose(pst[:], xf_src[:, j, ds(d * P, P)], cx.ident[:])
            nc.vector.tensor_copy(out=dst_bf[:, d, ds(j * P, P)], in_=pst[:])


def _rmsnorm_inplace(nc, cx, x_res, j, xr_t, nw_b):
    """x_res[:, j, :] = rmsnorm(xr_t) * nw_b  (xr_t: [128, D] f32 scratch)."""
    pool = cx.nrmp
    sq = pool.tile([P, D], F32, tag="nrm_sq")
    ssq = pool.tile([P, 1], F32, tag="nrm_ssq")
    rs = pool.tile([P, 1], F32, tag="nrm_rs")
    nc.vector.tensor_tensor(out=sq[:], in0=xr_t[:], in1=xr_t[:], op=MUL)
    nc.vector.tensor_reduce(out=ssq[:], in_=sq[:], axis=mybir.AxisListType.X, op=ADD)
    nc.scalar.activation(out=rs[:], in_=ssq[:], func=AF.Sqrt, scale=1.0 / D, bias=cx.eps_t[:, :1])
    nc.vector.reciprocal(out=rs[:], in_=rs[:])
    nc.scalar.mul(out=sq[:], in_=xr_t[:], mul=rs[:, :1])
    nc.vector.tensor_tensor(out=x_res[:, j, :], in0=sq[:], in1=nw_b[:], op=MUL)


def build():
    nc = bacc.Bacc("TRN2", target_bir_lowering=False, debug=False,
                   num_devices=NCORES)
    A = {}

    def inp(name, shape, dt):
        A[name] = nc.dram_tensor(name, list(shape), dt, kind="ExternalInput")
        return A[name]

    inp("idx", (TLOC, 1), I32)
    inp("etab", (TLOC, D), F32)
    inp("cosT", (TLOC, D // 2), F32)
    inp("sinT", (TLOC, D // 2), F32)
    if L_RUN > 0:
        inp("wq", (L_RUN, D, D), BF)
        inp("wk", (L_RUN, D, D), BF)
        inp("wv", (L_RUN, D, D), BF)
        inp("wo", (L_RUN, D, D), BF)
        inp("boB", (L_RUN, P, D), F32)
        inp("n1B", (L_RUN, P, D), F32)
        inp("wg", (L_RUN, D, FF), BF)
        inp("wu", (L_RUN, D, FF), BF)
        inp("wd", (L_RUN, FF, D), BF)
        inp("n2B", (L_RUN, P, D), F32)
    if STAGE in ("sb", "full"):
        inp("wsp", (D, D), BF)
        inp("bsp", (D, 1), F32)
        inp("memT", (D, SLOTS), BF)
        inp("memB", (SLOTS, D), BF)
        inp("wrp", (D, D), BF)
        inp("brpB", (P, D), F32)
        inp("noutB", (P, D), F32)
    if STAGE == "full":
        inp("et", (D, V), BF)
        out_t = nc.dram_tensor("logits", [TLOC, V], F32, kind="ExternalOutput")
    else:
        out_t = nc.dram_tensor("xdbg", [TLOC, D], F32, kind="ExternalOutput")

    with tile.TileContext(nc) as tc:
        _prog(nc, tc, A, out_t)
    nc.compile()
    return nc


def _layer(nc, tc, cx, A, l):
    ex = lambda **kw: tc.tile_pool(**kw)
    x_res = cx.x_res

    # ---------------- phase A: x->xT transposes, q/k/v projections, AllGather
    with ex(name=f"A{l}", bufs=1) as pA, \
         ex(name=f"Apt{l}", bufs=2, space="PSUM") as pt, \
         ex(name=f"Amm{l}", bufs=2, space="PSUM") as pmm:
        xT_bf = cx.xTp.tile([P, DT, TLOC], BF, tag="xT")
        _transpose_to_feat(nc, cx, pt, x_res, xT_bf)

        wq_t = pA.tile([P, DT, D], BF, tag="w", bufs=2)
        nc.sync.dma_start(wq_t[:], A["wq"].ap()[l].rearrange("(o p) n -> p o n", p=P))
        qT_bf = cx.qTp.tile([P, DT, TLOC], BF, tag="qT")
        for m in range(DT):
            pq = pmm.tile([P, 512], F32, tag="mm")
            for d in range(DT):
                nc.tensor.matmul(out=pq[:], lhsT=wq_t[:, d, ds(m * P, P)],
                                 rhs=xT_bf[:, d, :], start=(d == 0), stop=(d == DT - 1))
            nc.vector.tensor_copy(out=qT_bf[:, m, :], in_=pq[:])

        wk_t = pA.tile([P, DT, D], BF, tag="w", bufs=2)
        nc.sync.dma_start(wk_t[:], A["wk"].ap()[l].rearrange("(o p) n -> p o n", p=P))
        kT_loc = pA.tile([P, DT, TLOC], BF, tag="kT_loc")
        for m in range(DT):
            pk = pmm.tile([P, 512], F32, tag="mm")
            for d in range(DT):
                nc.tensor.matmul(out=pk[:], lhsT=wk_t[:, d, ds(m * P, P)],
                                 rhs=xT_bf[:, d, :], start=(d == 0), stop=(d == DT - 1))
            nc.vector.tensor_copy(out=kT_loc[:, m, :], in_=pk[:])

        wv_t = pA.tile([P, DT, D], BF, tag="w", bufs=2)
        nc.sync.dma_start(wv_t[:], A["wv"].ap()[l].rearrange("(o p) n -> p o n", p=P))
        v_loc = pA.tile([P, JT, D], BF, tag="v_loc")
        for j in range(JT):
            for n in range(NH):
                pv = pmm.tile([P, 512], F32, tag="mm")
                for d in range(DT):
                    nc.tensor.matmul(out=pv[:], lhsT=xT_bf[:, d, ds(j * P, P)],
                                     rhs=wv_t[:, d, ds(n * 512, 512)],
                                     start=(d == 0), stop=(d == DT - 1))
                nc.vector.tensor_copy(out=v_loc[:, j, ds(n * 512, 512)], in_=pv[:])

        kb_in = cx.dramp.tile([DT, P, TLOC], BF, tag="kb_in")
        nc.sync.dma_start(kb_in[:].rearrange("o p t -> p o t"), kT_loc[:])
        kb_out = cx.dramp.tile([4, DT, P, TLOC], BF, tag="kb_out")
        nc.gpsimd.collective_compute(
            "AllGather", mybir.AluOpType.bypass, replica_groups=GROUPS,
            ins=[kb_in[:].opt()], outs=[kb_out[:].opt()])
        vb_in = cx.dramp.tile([JT, P, D], BF, tag="vb_in")
        nc.sync.dma_start(vb_in[:].rearrange("j p n -> p j n"), v_loc[:])
        vb_out = cx.dramp.tile([4, JT, P, D], BF, tag="vb_out")
        nc.gpsimd.collective_compute(
            "AllGather", mybir.AluOpType.bypass, replica_groups=GROUPS,
            ins=[vb_in[:].opt()], outs=[vb_out[:].opt()])

    if SUBSTAGE == "A":
        return
    # ---------------- phase B: attention + out-proj + rmsnorm
    with ex(name=f"B{l}", bufs=1) as pB, \
         ex(name=f"Bmm{l}", bufs=2, space="PSUM") as pmm, \
         ex(name=f"Bav{l}", bufs=2, space="PSUM") as pav, \
         ex(name=f"Bsm{l}", bufs=2, space="PSUM") as psm:
        kT_full = pB.tile([P, DT, 4, TLOC], BF, tag="kT_full")
        v_full = pB.tile([P, KT, D], BF, tag="v_full")
        for r in range(4):
            nc.sync.dma_start(kT_full[:, :, r, :], kb_out[r].rearrange("o p t -> p o t"))
            nc.sync.dma_start(v_full[:, ds(r * JT, JT), :], vb_out[r].rearrange("j p n -> p j n"))
        wo_t = pB.tile([P, DT, D], BF, tag="wo")
        nc.sync.dma_start(wo_t[:], A["wo"].ap()[l].rearrange("(o p) n -> p o n", p=P))
        oT_bf = pB.tile([P, DT, TLOC], BF, tag="oT")

        for h in range(H):
            po = (h % 2) * DH
            mh = h // 2
            expT = pB.tile([P, KT, TLOC], BF, tag="expT")
            for kt in range(KT):
                ps_s = pmm.tile([P, 512], F32, tag="mm")
                nc.tensor.matmul(
                    out=ps_s[:],
                    lhsT=kT_full[ds(po, DH), mh, kt // JT, ds((kt % JT) * P, P)],
                    rhs=qT_bf[ds(po, DH), mh, :],
                    start=True, stop=True)
                nc.scalar.activation(out=expT[:, kt, :], in_=ps_s[:],
                                     func=AF.Exp, scale=DH ** -0.5)
            ps_se = psm.tile([1, 512], F32, tag="sm")
            for kt in range(KT):
                nc.tensor.matmul(out=ps_se[:], lhsT=cx.ones_bf[:, :1],
                                 rhs=expT[:, kt, :],
                                 start=(kt == 0), stop=(kt == KT - 1))
            rec = pB.tile([1, 512], F32, tag="rec", bufs=2)
            nc.vector.tensor_copy(out=rec[:], in_=ps_se[:])
            nc.vector.reciprocal(out=rec[:], in_=rec[:])
            recB = pB.tile([DH, 512], F32, tag="recB", bufs=2)
            nc.gpsimd.partition_broadcast(recB[:], rec[:1, :])
            ps_o = pav.tile([DH, 512], F32, tag="av")
            for kt in range(KT):
                nc.tensor.matmul(out=ps_o[:], lhsT=v_full[:, kt, ds(h * DH, DH)],
                                 rhs=expT[:, kt, :],
                                 start=(kt == 0), stop=(kt == KT - 1))
            nc.vector.tensor_tensor(out=oT_bf[ds(po, DH), mh, :],
                                    in0=ps_o[:], in1=recB[:], op=MUL)

        n1B_t = cx.nwp.tile([P, D], F32, tag="nw")
        nc.sync.dma_start(n1B_t[:], A["n1B"].ap()[l])
        boB_t = cx.nwp.tile([P, D], F32, tag="nw")
        nc.sync.dma_start(boB_t[:], A["boB"].ap()[l])
        xr_big = cx.xrp.tile([P, JT, D], F32, tag="xr")
        for j in range(JT):
            for n in range(NH):
                px = pmm.tile([P, 512], F32, tag="mm")
                for d in range(DT):
                    nc.tensor.matmul(out=px[:], lhsT=oT_bf[:, d, ds(j * P, P)],
                                     rhs=wo_t[:, d, ds(n * 512, 512)],
                                     start=(d == 0), stop=(d == DT - 1))
                nc.vector.tensor_tensor(out=xr_big[:, j, ds(n * 512, 512)],
                                        in0=px[:], in1=x_res[:, j, ds(n * 512, 512)], op=ADD)
            nc.vector.tensor_tensor(out=xr_big[:, j, :], in0=xr_big[:, j, :],
                                    in1=boB_t[:], op=ADD)
            _rmsnorm_inplace(nc, cx, x_res, j, xr_big[:, j, :], n1B_t)

    if SUBSTAGE == "B":
        return
    # ---------------- phase C: FFN
    with ex(name=f"C{l}", bufs=1) as pC, \
         ex(name=f"Cpt{l}", bufs=2, space="PSUM") as pt, \
         ex(name=f"Cmm{l}", bufs=2, space="PSUM") as pmm, \
         ex(name=f"Cacc{l}", bufs=1, space="PSUM") as pacc:
        xnT_bf = cx.xTp.tile([P, DT, TLOC], BF, tag="xT")
        _transpose_to_feat(nc, cx, pt, x_res, xnT_bf)
        su_full = pC.tile([P, FT, TLOC], BF, tag="su")
        wg_r = A["wg"].ap()[l].rearrange("(o p) f -> p o f", p=P)
        wu_r = A["wu"].ap()[l].rearrange("(o p) f -> p o f", p=P)
        for f in range(FT):
            wg_t = pC.tile([P, DT, P], BF, tag="wff", bufs=4)
            nc.sync.dma_start(wg_t[:], wg_r[:, :, ds(f * P, P)])
            wu_t = pC.tile([P, DT, P], BF, tag="wff", bufs=4)
            nc.sync.dma_start(wu_t[:], wu_r[:, :, ds(f * P, P)])
            ps_g = pmm.tile([P, 512], F32, tag="mm")
            for d in range(DT):
                nc.tensor.matmul(out=ps_g[:], lhsT=wg_t[:, d, :], rhs=xnT_bf[:, d, :],
                                 start=(d == 0), stop=(d == DT - 1))
            g_sb = pC.tile([P, 512], F32, tag="g_sb", bufs=2)
            nc.scalar.activation(out=g_sb[:], in_=ps_g[:], func=AF.Silu)
            ps_u = pmm.tile([P, 512], F32, tag="mm")
            for d in range(DT):
                nc.tensor.matmul(out=ps_u[:], lhsT=wu_t[:, d, :], rhs=xnT_bf[:, d, :],
                                 start=(d == 0), stop=(d == DT - 1))
            nc.vector.tensor_tensor(out=su_full[:, f, :], in0=g_sb[:], in1=ps_u[:], op=MUL)

        if SUBSTAGE == "C1":
            return
        n2B_t = cx.nwp.tile([P, D], F32, tag="nw")
        nc.sync.dma_start(n2B_t[:], A["n2B"].ap()[l])
        xr2 = cx.xrp.tile([P, JT, D], F32, tag="xr")
        for n in range(NH):
            ps_d = [pacc.tile([P, 512], F32, tag=f"acc{j}", name=f"ps_d{j}") for j in range(JT)]
            for f in range(FT):
                wd_t = pC.tile([P, 512], BF, tag="wd", bufs=3)
                nc.sync.dma_start(wd_t[:], A["wd"].ap()[l][ds(f * P, P), ds(n * 512, 512)])
                for j in range(JT):
                    nc.tensor.matmul(out=ps_d[j][:], lhsT=su_full[:, f, ds(j * P, P)],
                                     rhs=wd_t[:], start=(f == 0), stop=(f == FT - 1))
            for j in range(JT):
                nc.vector.tensor_tensor(out=xr2[:, j, ds(n * 512, 512)], in0=ps_d[j][:],
                                        in1=x_res[:, j, ds(n * 512, 512)], op=ADD)
        for j in range(JT):
            _rmsnorm_inplace(nc, cx, x_res, j, xr2[:, j, :], n2B_t)


def _statebank(nc, tc, cx, A):
    ex = lambda **kw: tc.tile_pool(**kw)
    x_res = cx.x_res
    with ex(name="S", bufs=1) as pS, \
         ex(name="Spt", bufs=2, space="PSUM") as pt, \
         ex(name="Smm", bufs=2, space="PSUM") as pmm:
        xT_bf = cx.xTp.tile([P, DT, TLOC], BF, tag="xT")
        _transpose_to_feat(nc, cx, pt, x_res, xT_bf)

        wsp_t = pS.tile([P, DT, D], BF, tag="wsp")
        nc.sync.dma_start(wsp_t[:], A["wsp"].ap().rearrange("(o p) n -> p o n", p=P))
        bsp_t = pS.tile([P, DT], F32, tag="bsp")
        nc.sync.dma_start(bsp_t[:], A["bsp"].ap().rearrange("(o p) u -> p (o u)", p=P))
        qspT = cx.qTp.tile([P, DT, TLOC], BF, tag="qT")
        for m in range(DT):
            ps_q = pmm.tile([P, 512], F32, tag="mm")
            for d in range(DT):
                nc.tensor.matmul(out=ps_q[:], lhsT=wsp_t[:, d, ds(m * P, P)],
                                 rhs=xT_bf[:, d, :], start=(d == 0), stop=(d == DT - 1))
            nc.scalar.activation(out=qspT[:, m, :], in_=ps_q[:], func=AF.Identity,
                                 bias=bsp_t[:, m:m + 1])

        expS = pS.tile([P, ST, TLOC], BF, tag="sexp")
        memT_r = A["memT"].ap().rearrange("(o p) s -> p o s", p=P)
        for s in range(ST):
            mt_t = pS.tile([P, DT, P], BF, tag="mt", bufs=4)
            nc.sync.dma_start(mt_t[:], memT_r[:, :, ds(s * P, P)])
            ps_s = pmm.tile([P, 512], F32, tag="mm")
            for d in range(DT):
                nc.tensor.matmul(out=ps_s[:], lhsT=mt_t[:, d, :], rhs=qspT[:, d, :],
                                 start=(d == 0), stop=(d == DT - 1))
            nc.scalar.activation(out=expS[:, s, :], in_=ps_s[:], func=AF.Exp,
                                 scale=D ** -0.5)

        rsum = pS.tile([P, JT], F32, tag="rsum")
        with ex(name="Ssm", bufs=2, space="PSUM") as psm:
            for j in range(JT):
                ps_rs = psm.tile([P, 1], F32, tag="sm")
                for s in range(ST):
                    nc.tensor.matmul(out=ps_rs[:], lhsT=expS[:, s, ds(j * P, P)],
                                     rhs=cx.ones_bf[:, :1], start=(s == 0), stop=(s == ST - 1))
                nc.vector.tensor_copy(out=rsum[:, j:j + 1], in_=ps_rs[:])
        nc.vector.reciprocal(out=rsum[:], in_=rsum[:])

        r_tok = pS.tile([P, JT, D], F32, tag="r_tok")
        with ex(name="Sacc", bufs=1, space="PSUM") as pacc:
            for n in range(NH):
                ps_r = [pacc.tile([P, 512], F32, tag=f"acc{j}", name=f"ps_r{j}") for j in range(JT)]
                for s in range(ST):
                    mb_t = pS.tile([P, 512], BF, tag="mb", bufs=4)
                    nc.sync.dma_start(mb_t[:], A["memB"].ap()[ds(s * P, P), ds(n * 512, 512)])
                    for j in range(JT):
                        nc.tensor.matmul(out=ps_r[j][:], lhsT=expS[:, s, ds(j * P, P)],
                                         rhs=mb_t[:], start=(s == 0), stop=(s == ST - 1))
                for j in range(JT):
                    nc.scalar.mul(out=r_tok[:, j, ds(n * 512, 512)], in_=ps_r[j][:],
                                  mul=rsum[:, j:j + 1])

        rT_bf = pS.tile([P, DT, TLOC], BF, tag="rT")
        _transpose_to_feat(nc, cx, pt, r_tok, rT_bf)
        wrp_t = pS.tile([P, DT, D], BF, tag="wsp")
        nc.sync.dma_start(wrp_t[:], A["wrp"].ap().rearrange("(o p) n -> p o n", p=P))
        brpB_t = pS.tile([P, D], F32, tag="brpB")
        nc.sync.dma_start(brpB_t[:], A["brpB"].ap())
        noutB_t = pS.tile([P, D], F32, tag="noutB")
        nc.sync.dma_start(noutB_t[:], A["noutB"].ap())
        xr3 = cx.xrp.tile([P, JT, D], F32, tag="xr")
        for j in range(JT):
            for n in range(NH):
                px = pmm.tile([P, 512], F32, tag="mm")
                for d in range(DT):
                    nc.tensor.matmul(out=px[:], lhsT=rT_bf[:, d, ds(j * P, P)],
                                     rhs=wrp_t[:, d, ds(n * 512, 512)],
                                     start=(d == 0), stop=(d == DT - 1))
                nc.vector.tensor_tensor(out=xr3[:, j, ds(n * 512, 512)], in0=px[:],
                                        in1=x_res[:, j, ds(n * 512, 512)], op=ADD)
            nc.vector.tensor_tensor(out=xr3[:, j, :], in0=xr3[:, j, :], in1=brpB_t[:], op=ADD)
            _rmsnorm_inplace(nc, cx, x_res, j, xr3[:, j, :], noutB_t)


def _lm_head(nc, tc, cx, A, out_t):
    ex = lambda **kw: tc.tile_pool(**kw)
    NV = (V + 511) // 512  # 63: 62*512 + 256
    with ex(name="LM", bufs=1) as pL, \
         ex(name="Lpt", bufs=2, space="PSUM") as pt, \
         ex(name="Lacc", bufs=6, space="PSUM") as pacc:
        hT_bf = pL.tile([P, DT, TLOC], BF, tag="hT")
        _transpose_to_feat(nc, cx, pt, cx.x_res, hT_bf)
        et_r = A["et"].ap().rearrange("(o p) v -> p o v", p=P)
        for vt in range(NV):
            nv = 512 if vt < NV - 1 else V - 512 * (NV - 1)
            et_t = pL.tile([P, DT, 512], BF, tag="et", bufs=3)
            nc.sync.dma_start(et_t[:, :, :nv], et_r[:, :, ds(vt * 512, nv)])
            for j in range(JT):
                ps = pacc.tile([P, 512], F32, tag="acc")
                for d in range(DT):
                    nc.tensor.matmul(out=ps[:, :nv], lhsT=hT_bf[:, d, ds(j * P, P)],
                                     rhs=et_t[:, d, :nv], start=(d == 0), stop=(d == DT - 1))
                lg_sb = pL.tile([P, 512], F32, tag="lg", bufs=4)
                nc.vector.tensor_copy(out=lg_sb[:, :nv], in_=ps[:, :nv])
                nc.sync.dma_start(out_t.ap()[ds(j * P, P), ds(vt * 512, nv)], lg_sb[:, :nv])


def _prog(nc, tc, A, out_t):
    ex = lambda **kw: tc.tile_pool(**kw)
    cx = _Ctx()
    with ex(name="const", bufs=1) as constp, \
         ex(name="xres", bufs=1) as xresp, \
         ex(name="xT", bufs=1) as cx.xTp, \
         ex(name="qT", bufs=1) as cx.qTp, \
         ex(name="nw", bufs=2) as cx.nwp, \
         ex(name="xr", bufs=1) as cx.xrp, \
         ex(name="nrm", bufs=2) as cx.nrmp, \
         ex(name="dram", bufs=2, space="DRAM") as cx.dramp:
        cx.ident = constp.tile([P, P], F32)
        make_identity(nc, cx.ident[:])
        cx.ones_bf = constp.tile([P, 1], BF)
        nc.any.memset(cx.ones_bf[:], 1.0)
        cx.eps_t = constp.tile([P, 1], F32)
        nc.any.memset(cx.eps_t[:], EPS)
        cx.x_res = xresp.tile([P, JT, D], F32)
        x_res = cx.x_res

        # ------------------------------------------------- embedding + rope
        with ex(name="emb", bufs=2) as embp, ex(name="embi", bufs=1) as embip:
            idx_sb = embip.tile([P, JT], I32)
            nc.sync.dma_start(idx_sb[:], A["idx"].ap().rearrange("(o p) u -> p (o u)", p=P))
            for j in range(JT):
                x0 = embp.tile([P, D], F32, tag="x0")
                nc.gpsimd.indirect_dma_start(
                    out=x0[:], out_offset=None, in_=A["etab"].ap(),
                    in_offset=bass.IndirectOffsetOnAxis(ap=idx_sb[:, j:j + 1], axis=0))
                cos_t = embp.tile([P, D // 2], F32, tag="cos")
                nc.sync.dma_start(cos_t[:], A["cosT"].ap()[ds(j * P, P), :])
                sin_t = embp.tile([P, D // 2], F32, tag="sin")
                nc.sync.dma_start(sin_t[:], A["sinT"].ap()[ds(j * P, P), :])
                x0v = x0[:].rearrange("p (i two) -> p i two", two=2)
                xrv = x_res[:, j, :].rearrange("p (i two) -> p i two", two=2)
                t1 = embp.tile([P, D // 2], F32, tag="t1")
                t2 = embp.tile([P, D // 2], F32, tag="t2")
                t3 = embp.tile([P, D // 2], F32, tag="t3")
                t4 = embp.tile([P, D // 2], F32, tag="t4")
                nc.vector.tensor_tensor(out=t1[:], in0=x0v[:, :, 0], in1=cos_t[:], op=MUL)
                nc.vector.tensor_tensor(out=t2[:], in0=x0v[:, :, 1], in1=sin_t[:], op=MUL)
                nc.vector.tensor_tensor(out=xrv[:, :, 0], in0=t1[:], in1=t2[:], op=SUB)
                nc.vector.tensor_tensor(out=t3[:], in0=x0v[:, :, 0], in1=sin_t[:], op=MUL)
                nc.vector.tensor_tensor(out=t4[:], in0=x0v[:, :, 1], in1=cos_t[:], op=MUL)
                nc.vector.tensor_tensor(out=xrv[:, :, 1], in0=t3[:], in1=t4[:], op=ADD)

        if STAGE == "emb":
            nc.sync.dma_start(out_t.ap().rearrange("(o p) d -> p o d", p=P), x_res[:])
            return

        for l in range(L_RUN):
            _layer(nc, tc, cx, A, l)

        if STAGE == "layers":
            nc.sync.dma_start(out_t.ap().rearrange("(o p) d -> p o d", p=P), x_res[:])
            return

        _statebank(nc, tc, cx, A)

        if STAGE == "sb":
            nc.sync.dma_start(out_t.ap().rearrange("(o p) d -> p o d", p=P), x_res[:])
            return

        _lm_head(nc, tc, cx, A, out_t)


# ----------------------------------------------------------------------------
# host-side input prep
# ----------------------------------------------------------------------------

def _bf(x):
    return np.ascontiguousarray(np.asarray(x, dtype=np.float32).astype(ml_dtypes.bfloat16))


def _f32(x):
    return np.ascontiguousarray(np.asarray(x, dtype=np.float32))


def _prep(inputs):
    ids = np.asarray(inputs["ids"]).astype(np.int64)
    tok_embed = np.asarray(inputs["tok_embed"], dtype=np.float32)

    def bB(a):
        a = np.asarray(a, np.float32)
        return np.ascontiguousarray(np.broadcast_to(a[:, None, :], (a.shape[0], P, D)))

    shared = {}
    if L_RUN > 0:
        shared.update({
            "wq": _bf(inputs["Wq"][:L_RUN]),
            "wk": _bf(inputs["Wk"][:L_RUN]),
            "wv": _bf(inputs["Wv"][:L_RUN]),
            "wo": _bf(inputs["Wo"][:L_RUN]),
            "boB": bB(np.asarray(inputs["bo"], np.float32)[:L_RUN]),
            "n1B": bB(np.asarray(inputs["n1"], np.float32)[:L_RUN]),
            "wg": _bf(inputs["Wg"][:L_RUN]),
            "wu": _bf(inputs["Wu"][:L_RUN]),
            "wd": _bf(inputs["Wd"][:L_RUN]),
            "n2B": bB(np.asarray(inputs["n2"], np.float32)[:L_RUN]),
        })
    if STAGE in ("sb", "full"):
        mem = np.asarray(inputs["mem"], np.float32)
        shared.update({
            "wsp": _bf(inputs["Wsp"]),
            "bsp": _f32(np.asarray(inputs["bsp"]).reshape(D, 1)),
            "memT": _bf(mem.T),
            "memB": _bf(mem),
            "wrp": _bf(inputs["Wrp"]),
            "brpB": _f32(np.broadcast_to(np.asarray(inputs["brp"], np.float32)[None, :], (P, D))),
            "noutB": _f32(np.broadcast_to(np.asarray(inputs["nout"], np.float32)[None, :], (P, D))),
        })
    if STAGE == "full":
        shared["et"] = _bf(tok_embed.T)

    inv_freq = 1.0 / (10000.0 ** (np.arange(0, D, 2, dtype=np.float32) / D))
    percore = []
    for c in range(NCORES):
        b, s = c // 4, c % 4
        tok = ids[b, s * TLOC:(s + 1) * TLOC]
        uids, invm = np.unique(tok, return_inverse=True)
        etab = np.zeros((TLOC, D), np.float32)
        etab[:len(uids)] = tok_embed[uids]
        pos = (s * TLOC + np.arange(TLOC, dtype=np.float32))[:, None]
        freqs = pos * inv_freq[None, :]
        percore.append({
            "idx": invm.astype(np.int32).reshape(TLOC, 1),
            "etab": etab,
            "cosT": np.cos(freqs).astype(np.float32),
            "sinT": np.sin(freqs).astype(np.float32),
        })
    return shared, percore


# ----------------------------------------------------------------------------
# runner: shared arrays uploaded once + device-to-device fanout
# ----------------------------------------------------------------------------

def _run_fast(nc, shared, percore, n_cores=NCORES):
    import jax
    from jax.sharding import Mesh, PartitionSpec, NamedSharding
    from jax.experimental.shard_map import shard_map
    from concourse import bass2jax

    bass2jax.install_neuronx_cc_hook()
    devs = jax.devices()[:n_cores]
    mesh = Mesh(np.asarray(devs), ("core",))
    spec = PartitionSpec("core")
    shd = NamedSharding(mesh, spec)

    placed = {}
    for name, arr in shared.items():
        a0 = jax.device_put(arr, devs[0])
        a0.block_until_ready()
        reps = [a0] + [jax.device_put(a0, d) for d in devs[1:]]
        gshape = (n_cores * arr.shape[0], *arr.shape[1:])
        placed[name] = jax.make_array_from_single_device_arrays(gshape, shd, reps)
    for name in percore[0]:
        arrs = [jax.device_put(percore[c][name], devs[c]) for c in range(n_cores)]
        a = percore[0][name]
        gshape = (n_cores * a.shape[0], *a.shape[1:])
        placed[name] = jax.make_array_from_single_device_arrays(gshape, shd, arrs)

    partition_name = nc.partition_id_tensor.name if nc.partition_id_tensor else None
    in_names, out_names, out_avals = [], [], []
    for alloc in nc.m.functions[0].allocations:
        if not isinstance(alloc, mybir.MemoryLocationSet):
            continue
        name = alloc.memorylocations[0].name
        if alloc.kind == "ExternalInput":
            if name != partition_name:
                in_names.append(name)
        elif alloc.kind == "ExternalOutput":
            out_names.append(name)
            out_avals.append(jax.core.ShapedArray(tuple(alloc.tensor_shape),
                                                  mybir.dt.np(alloc.dtype)))
    n_params = len(in_names)
    all_in_names = list(in_names) + list(out_names)
    if partition_name is not None:
        all_in_names.append(partition_name)

    def _body(*args):
        operands = list(args)
        if partition_name is not None:
            operands.append(bass2jax.partition_id_tensor())
        outs = bass2jax._bass_exec_p.bind(
            *operands,
            out_avals=tuple(out_avals),
            in_names=tuple(all_in_names),
            out_names=tuple(out_names),
            lowering_input_output_aliases=(),
            sim_require_finite=True,
            sim_require_nnan=True,
            nc=nc,
        )
        return tuple(outs)

    profile_req = bool(os.environ.get("DNC_PROF"))
    donate = tuple(range(n_params, n_params + len(out_names)))
    sharded = jax.jit(
        shard_map(_body, mesh=mesh, in_specs=(spec,) * (n_params + len(out_names)),
                  out_specs=(spec,) * len(out_names), check_rep=False),
        donate_argnums=donate, keep_unused=True)

    def mk_zeros():
        return [
            jax.jit(lambda av=av: jax.numpy.zeros((n_cores * av.shape[0], *av.shape[1:]), av.dtype),
                    out_shardings=shd)()
            for av in out_avals
        ]

    args = [placed[name] for name in in_names] + mk_zeros()
    out_arrs = sharded(*args)
    if profile_req:
        [o.block_until_ready() for o in out_arrs]
        _profile_exec(nc, lambda: sharded(*([placed[name] for name in in_names] + mk_zeros())))
    res = []
    for c in range(n_cores):
        res.append({
            name: np.asarray(out_arrs[i]).reshape(n_cores, *out_avals[i].shape)[c]
            for i, name in enumerate(out_names)
        })
    return res


def _profile_exec(nc, run_fn):
    """Re-run the jitted NEFF under the axon NTFF hook; print HW exec time."""
    import glob as _glob
    import tempfile
    try:
        from antenv.axon_hooks import get_axon_ntff_profile_hook
        hook = get_axon_ntff_profile_hook()
        if hook is None:
            print("HW exec time: unavailable (no ntff hook)")
            return
        import gauge.profiler
        from concourse import bass_utils as BU
        from concourse._compat import FishPath
        tmpdir = tempfile.mkdtemp(prefix="dnc_prof_")
        with hook(tmpdir, [0]):
            outs = run_fn()
            [o.block_until_ready() for o in outs]
        ntffs = _glob.glob(os.path.join(tmpdir, "*_body*.ntff"))
        if not ntffs:
            print("HW exec time: unavailable (no ntff produced)")
            return
        profile = gauge.profiler.Profile(
            profile_path=FishPath(tmpdir), kernel_dev_mode=True,
            profile_on_exit=False, bass_kernel=nc.m, offline_processing=True,
            fname="*_body*", metadata={"artifacts_path": tmpdir})
        res = BU._process_ntff_profile(
            profile, tmpdir, nc, list(range(NCORES)), None, False, {},
            trace_events=False)
        print(f"HW exec time: {res.exec_time_ns} ns")
    except Exception as e:
        import traceback; traceback.print_exc()
        print(f"HW exec time: error ({e})")


def _run_plain(nc, shared, percore, **kw):
    from concourse.bass_utils import run_bass_kernel_spmd
    in_maps = [dict(shared, **percore[c]) for c in range(NCORES)]
    return run_bass_kernel_spmd(nc, in_maps, core_ids=list(range(NCORES)), **kw)


_NC_CACHE = {}


def _get_nc():
    key = (L_RUN, STAGE)
    if key not in _NC_CACHE:
        _NC_CACHE[key] = build()
    return _NC_CACHE[key]


def _assemble(res, name, width):
    out = np.empty((B, T, width), np.float32)
    for c in range(NCORES):
        b, s = c // 4, c % 4
        out[b, s * TLOC:(s + 1) * TLOC, :] = res[c][name]
    return out


def kernel(**inputs):
    nc = _get_nc()
    shared, percore = _prep(inputs)
    if os.environ.get("DNC_PLAIN"):
        res = _run_plain(nc, shared, percore).results
    else:
        res = _run_fast(nc, shared, percore)
    if STAGE == "full":
        return _assemble(res, "logits", V)
    return _assemble(res, "xdbg", D)
